# revision 7
# baseline (speedup 1.0000x reference)
"""Trainium2 Bass kernel for nn_BatchedTrilLinear.

y[n, b*64:(b+1)*64] = x[n, b*64:(b+1)*64] @ L_b.T  for b in range(512),
with L_b = tril(W_b, -1) + diag(exp(diag(W_b))).

Sharding: data-parallel on N — each of the 8 cores processes a contiguous
512-row slice of x (zero-copy views) with the full 8 MB weights replicated.

Per-core dataflow (all fp32; matmuls in float32r = full-rate fp32 mode):
  - x loaded in natural layout with 2 MB DMAs (group of 8 strips x 4 n-tiles)
  - weights transformed on-chip (tril mask + exp(diag)) and transposed per
    64x64 block via PE into the stationary layout LT[64j+i, s, o] = L_{2s+j}[o, i]
  - per strip (128 columns = 2 blocks):
      PE-transpose x chunks -> xT [128 (j,i), 512 n] (one psum bank),
      2 concurrent quadrant matmuls (tile_position (0,0)/(64,64)) -> yT psum,
      PE-transpose back -> y natural chunks, copy into output group buffer
  - y stored with 2 MB DMAs
"""
import os
import sys
from contextlib import ExitStack

for _p in ("/opt/trn_rl_repo",):
    if os.path.isdir(_p) and _p not in sys.path:
        sys.path.insert(0, _p)

import numpy as np

N_FULL = 4096
B_FULL = 512
D = 64
NCORES = 8
NS = N_FULL // NCORES        # rows per core

_built = {}


def _body(ctx, tc, y_d, x_d, w_d, *, NS, B, SG, SC, use_f32r):
    import concourse.mybir as mybir
    from concourse.masks import make_identity

    nc = tc.nc
    f32 = mybir.dt.float32
    f32r = mybir.dt.float32r
    dt_mm = f32r if use_f32r else f32
    S = B // 2               # strips (2 blocks each)
    NT = NS // 128           # n-tiles
    G = S // SG              # strip groups
    CG = SG * 128            # columns per group
    WC = S // SC             # weight-prep chunks
    GPC = G // WC            # groups per weight chunk

    const_pool = ctx.enter_context(tc.tile_pool(name="const", bufs=1))
    wp = ctx.enter_context(tc.tile_pool(name="wp", bufs=2))
    wpsum = ctx.enter_context(tc.tile_pool(name="wpsum", bufs=2, space="PSUM"))
    lt_pool = ctx.enter_context(tc.tile_pool(name="lt", bufs=1))
    xg_pool = ctx.enter_context(tc.tile_pool(name="xg", bufs=2))
    yg_pool = ctx.enter_context(tc.tile_pool(name="yg", bufs=2))
    xt_pool = ctx.enter_context(tc.tile_pool(name="xt", bufs=3))
    yt_pool = ctx.enter_context(tc.tile_pool(name="yt", bufs=3))
    bd_pool = ctx.enter_context(tc.tile_pool(name="bd", bufs=3))
    psx_pool = ctx.enter_context(tc.tile_pool(name="psx", bufs=2, space="PSUM"))
    psy_pool = ctx.enter_context(tc.tile_pool(name="psy", bufs=2, space="PSUM"))
    psyt_pool = ctx.enter_context(tc.tile_pool(name="psyt", bufs=2, space="PSUM"))

    ident = const_pool.tile([128, 128], f32)
    make_identity(nc, ident)

    # block-diagonal 0/1 mask [128, 128] in matmul dtype (f32r needs a
    # rounded producer, so build in fp32 and copy-cast once)
    bdm_f = const_pool.tile([128, 128], f32)
    nc.gpsimd.memset(bdm_f[:], 0.0)
    for h in range(2):
        nc.gpsimd.memset(bdm_f[64 * h:64 * h + 64, 64 * h:64 * h + 64], 1.0)
    bdm = const_pool.tile([128, 2, 64], dt_mm)
    nc.vector.tensor_copy(bdm[:], bdm_f.rearrange("p (u c) -> p u c", c=64))

    # masks [128, D]: partition p = 64*j + o, free = i
    tril_m = const_pool.tile([128, D], f32)   # 1 if i < o (strictly lower)
    diag_m = const_pool.tile([128, D], f32)   # 1 if i == o
    for h in range(2):
        tsl = tril_m[64 * h:64 * h + 64, :]
        nc.gpsimd.memset(tsl, 1.0)
        nc.gpsimd.affine_select(out=tsl, in_=tsl, compare_op=mybir.AluOpType.is_gt,
                                fill=0.0, base=0, pattern=[[-1, D]],
                                channel_multiplier=1)
        dsl = diag_m[64 * h:64 * h + 64, :]
        nc.gpsimd.memset(dsl, 0.0)
        nc.gpsimd.affine_select(out=dsl, in_=dsl,
                                compare_op=mybir.AluOpType.not_equal,
                                fill=1.0, base=0, pattern=[[-1, D]],
                                channel_multiplier=1)

    w_view = w_d.rearrange("(s j) o i -> j o s i", j=2)   # [2, 64, S, 64]
    x_view = x_d.rearrange("(t p) c -> p t c", p=128)     # [128, NT, C]
    y_view = y_d.rearrange("(t p) c -> p t c", p=128)

    lt_tiles = []

    def prep_weight_chunk(c):
        """Build LT chunk c: LT[64j+i, sl, o] = L_{2(c*SC+sl)+j}[o, i]."""
        wr = wp.tile([128, SC, D], f32, tag="wr")
        for j in range(2):
            nc.sync.dma_start(wr[64 * j:64 * j + 64],
                              w_view[j, :, c * SC:(c + 1) * SC, :])
        shp = (128, SC, D)
        tmp = wp.tile(list(shp), f32, tag="wtmp")
        nc.vector.tensor_tensor(tmp[:], wr[:], diag_m[:, None, :].to_broadcast(shp),
                                op=mybir.AluOpType.mult)
        dsum = wp.tile([128, SC], f32, tag="dsum")
        nc.vector.reduce_sum(dsum[:], tmp[:], axis=mybir.AxisListType.X)
        dexp = wp.tile([128, SC], f32, tag="dexp")
        nc.scalar.activation(dexp[:], dsum[:], mybir.ActivationFunctionType.Exp)
        nc.vector.tensor_tensor(wr[:], wr[:], tril_m[:, None, :].to_broadcast(shp),
                                op=mybir.AluOpType.mult)
        nc.vector.tensor_tensor(tmp[:], diag_m[:, None, :].to_broadcast(shp),
                                dexp[:, :, None].to_broadcast(shp),
                                op=mybir.AluOpType.mult)
        nc.vector.tensor_tensor(wr[:], wr[:], tmp[:], op=mybir.AluOpType.add)

        # Walrus requires transpose outputs at PSUM partition 0, so stage the
        # two 64x64 blocks on the diagonal of a 128x128 tile and do one full
        # transpose; the off-diagonal garbage is never read back.
        lt = lt_pool.tile([128, SC, D], dt_mm, tag=f"lt{c}")
        for sl in range(SC):
            stage = wp.tile([128, 128], f32, tag="wstage")
            nc.scalar.copy(stage[0:64, 0:64], wr[0:64, sl, :])
            nc.scalar.copy(stage[64:128, 64:128], wr[64:128, sl, :])
            pslt = wpsum.tile([128, 128], f32, tag="pslt")
            nc.tensor.matmul(pslt[:], lhsT=stage[:], rhs=ident[:],
                             is_transpose=True)
            nc.vector.tensor_copy(lt[0:64, sl, :], pslt[0:64, 0:64])
            nc.vector.tensor_copy(lt[64:128, sl, :], pslt[64:128, 64:128])
        lt_tiles.append(lt)
        return lt

    def do_group(g):
        xg = xg_pool.tile([128, NT, CG], f32, tag="xg")
        nc.sync.dma_start(xg[:], x_view[:, :, g * CG:(g + 1) * CG])
        yg = yg_pool.tile([128, NT, CG], f32, tag="yg")
        for sl in range(SG):
            s = g * SG + sl
            lt = lt_tiles[s // SC]
            slc = s % SC
            # x chunks -> xT strip [128 (j,i), NS]
            psx = psx_pool.tile([128, NS], f32, tag="psx")
            for t in range(NT):
                nc.tensor.matmul(psx[:, t * 128:(t + 1) * 128],
                                 lhsT=xg[:, t, sl * 128:(sl + 1) * 128],
                                 rhs=ident[:], is_transpose=True,
                                 start=(t == 0), stop=(t == NT - 1))
            xt = xt_pool.tile([128, NS], dt_mm, tag="xt")
            nc.scalar.copy(xt[:], psx[:])
            # fp32-family matmuls need PSUM partition 0, so use one full
            # 128-contraction matmul with a block-diagonal stationary:
            # bd[64j+i, 64j'+o] = LT[64j+i, o] * blockdiag_mask
            bd = bd_pool.tile([128, 2, 64], dt_mm, tag="bd")
            nc.vector.tensor_tensor(
                bd[:], lt[:, slc, None, :].to_broadcast((128, 2, D)),
                bdm[:], op=mybir.AluOpType.mult)
            psy = psy_pool.tile([128, NS], f32, tag="psy")
            nc.tensor.matmul(psy[:], lhsT=bd.rearrange("p u c -> p (u c)"),
                             rhs=xt[:])
            yt = yt_pool.tile([128, NS], f32, tag="yt")
            nc.vector.tensor_copy(yt[:], psy[:])
            # back to natural layout
            psyt = psyt_pool.tile([128, NS], f32, tag="psyt")
            for t in range(NT):
                nc.tensor.matmul(psyt[:, t * 128:(t + 1) * 128],
                                 lhsT=yt[:, t * 128:(t + 1) * 128],
                                 rhs=ident[:], is_transpose=True,
                                 start=(t == 0), stop=(t == NT - 1))
            dst = yg[:, :, sl * 128:(sl + 1) * 128]
            src = psyt.rearrange("p (t c) -> p t c", c=128)
            if sl % 2 == 0:
                nc.scalar.copy(dst, src)
            else:
                nc.vector.tensor_copy(dst, src)
        nc.sync.dma_start(y_view[:, :, g * CG:(g + 1) * CG], yg[:])

    # interleave weight-chunk prep with the strip groups that consume it
    for c in range(WC):
        prep_weight_chunk(c)
        for g in range(c * GPC, (c + 1) * GPC):
            do_group(g)


def build(NS=NS, B=B_FULL, SG=8, SC=16, use_f32r=True):
    key = (NS, B, SG, SC, use_f32r)
    if key in _built:
        return _built[key]
    import concourse.tile as tile
    import concourse.mybir as mybir
    from concourse import bacc

    f32 = mybir.dt.float32
    C = B * D
    nc = bacc.Bacc("TRN2", target_bir_lowering=False, debug=False)
    x_d = nc.dram_tensor("x", [NS, C], f32, kind="ExternalInput").ap()
    w_d = nc.dram_tensor("w", [B, D, D], f32, kind="ExternalInput").ap()
    y_d = nc.dram_tensor("y", [NS, C], f32, kind="ExternalOutput").ap()
    with tile.TileContext(nc) as tc, ExitStack() as ctx:
        _body(ctx, tc, y_d, x_d, w_d, NS=NS, B=B, SG=SG, SC=SC,
              use_f32r=use_f32r)
    nc.compile()
    _built[key] = nc
    return nc


def run(x, weights, trace=False, use_f32r=True):
    from concourse import bass_utils

    x = np.asarray(x)
    weights = np.asarray(weights)
    assert x.shape == (N_FULL, B_FULL * D), x.shape
    assert weights.shape == (B_FULL, D, D), weights.shape
    x32 = np.ascontiguousarray(x, dtype=np.float32)
    w32 = np.ascontiguousarray(weights, dtype=np.float32)

    nc = build(use_f32r=use_f32r)
    in_maps = [{"x": x32[k * NS:(k + 1) * NS], "w": w32} for k in range(NCORES)]
    res = bass_utils.run_bass_kernel_spmd(
        nc, in_maps, core_ids=list(range(NCORES)), trace=trace)
    y = np.concatenate([res.results[k]["y"] for k in range(NCORES)], axis=0)
    return y.astype(x.dtype, copy=False), res


def kernel(x, weights):
    y, _ = run(x, weights)
    return y


# revision 14
# speedup vs baseline: 24.5692x; 24.5692x over previous
"""Trainium2 Bass kernel for nn_BatchedTrilLinear.

y[n, b*64:(b+1)*64] = x[n, b*64:(b+1)*64] @ L_b.T  for b in range(512),
with L_b = tril(W_b, -1) + diag(exp(diag(W_b))).

Sharding: data-parallel on N — each of the 8 cores processes a contiguous
512-row slice of x (zero-copy views) with the full 8 MB weights replicated.

Per-core dataflow (all fp32; matmuls in float32r = full-rate fp32 mode):
  - x loaded in natural layout with 2 MB DMAs (group of 8 strips x 4 n-tiles)
  - weights transformed on-chip (tril mask + exp(diag)) and transposed per
    64x64 block via PE into the stationary layout LT[64j+i, s, o] = L_{2s+j}[o, i]
  - per strip (128 columns = 2 blocks):
      PE-transpose x chunks -> xT [128 (j,i), 512 n] (one psum bank),
      2 concurrent quadrant matmuls (tile_position (0,0)/(64,64)) -> yT psum,
      PE-transpose back -> y natural chunks, copy into output group buffer
  - y stored with 2 MB DMAs
"""
import os
import sys
from contextlib import ExitStack

for _p in ("/opt/trn_rl_repo",):
    if os.path.isdir(_p) and _p not in sys.path:
        sys.path.insert(0, _p)

import numpy as np

N_FULL = 4096
B_FULL = 512
D = 64
NCORES = 8
NS = N_FULL // NCORES        # rows per core

_built = {}


def _body(ctx, tc, y_d, x_d, w_d, *, NS, B, SG, SC, use_f32r, f32r_t=False,
          repeat=1):
    import concourse.mybir as mybir
    from concourse.masks import make_identity

    nc = tc.nc
    f32 = mybir.dt.float32
    f32r = mybir.dt.float32r
    dt_mm = f32r if use_f32r else f32
    dt_t = f32r if f32r_t else f32   # dtype along the transpose paths
    S = B // 2               # strips (2 blocks each)
    NT = NS // 128           # n-tiles
    G = S // SG              # strip groups
    CG = SG * 128            # columns per group
    WC = S // SC             # weight-prep chunks
    GPC = G // WC            # groups per weight chunk

    const_pool = ctx.enter_context(tc.tile_pool(name="const", bufs=1))
    wp = ctx.enter_context(tc.tile_pool(name="wp", bufs=2))
    wpsum = ctx.enter_context(tc.tile_pool(name="wpsum", bufs=2, space="PSUM"))
    lt_pool = ctx.enter_context(tc.tile_pool(name="lt", bufs=3))
    xg_pool = ctx.enter_context(tc.tile_pool(name="xg", bufs=2))
    yg_pool = ctx.enter_context(tc.tile_pool(name="yg", bufs=2))
    xt_pool = ctx.enter_context(tc.tile_pool(name="xt", bufs=3))
    yt_pool = ctx.enter_context(tc.tile_pool(name="yt", bufs=3))
    bd_pool = ctx.enter_context(tc.tile_pool(name="bd", bufs=3))
    psx_pool = ctx.enter_context(tc.tile_pool(name="psx", bufs=2, space="PSUM"))
    psy_pool = ctx.enter_context(tc.tile_pool(name="psy", bufs=2, space="PSUM"))
    psyt_pool = ctx.enter_context(tc.tile_pool(name="psyt", bufs=2, space="PSUM"))

    ident = const_pool.tile([128, 128], f32)
    make_identity(nc, ident)
    if f32r_t:
        ident_t = const_pool.tile([128, 128], f32r)
        nc.vector.tensor_copy(ident_t[:], ident[:])
    else:
        ident_t = ident

    # block-diagonal 0/1 mask [128, 128] in matmul dtype (f32r needs a
    # rounded producer, so build in fp32 and copy-cast once)
    bdm_f = const_pool.tile([128, 128], f32)
    nc.gpsimd.memset(bdm_f[:], 0.0)
    for h in range(2):
        nc.gpsimd.memset(bdm_f[64 * h:64 * h + 64, 64 * h:64 * h + 64], 1.0)
    bdm = const_pool.tile([128, 2, 64], dt_mm)
    nc.vector.tensor_copy(bdm[:], bdm_f.rearrange("p (u c) -> p u c", c=64))

    # masks [128, D]: partition p = 64*j + o, free = i
    tril_m = const_pool.tile([128, D], f32)   # 1 if i < o (strictly lower)
    diag_m = const_pool.tile([128, D], f32)   # 1 if i == o
    for h in range(2):
        tsl = tril_m[64 * h:64 * h + 64, :]
        nc.gpsimd.memset(tsl, 1.0)
        nc.gpsimd.affine_select(out=tsl, in_=tsl, compare_op=mybir.AluOpType.is_gt,
                                fill=0.0, base=0, pattern=[[-1, D]],
                                channel_multiplier=1)
        dsl = diag_m[64 * h:64 * h + 64, :]
        nc.gpsimd.memset(dsl, 0.0)
        nc.gpsimd.affine_select(out=dsl, in_=dsl,
                                compare_op=mybir.AluOpType.not_equal,
                                fill=1.0, base=0, pattern=[[-1, D]],
                                channel_multiplier=1)

    w_view = w_d.rearrange("(s j) o i -> j o s i", j=2)   # [2, 64, S, 64]
    x_view = x_d.rearrange("(t p) c -> p t c", p=128)     # [128, NT, C]
    y_view = y_d.rearrange("(t p) c -> p t c", p=128)

    lt_tiles = []

    def prep_weight_chunk(c):
        """Build LT chunk c: LT[64j+i, sl, o] = L_{2(c*SC+sl)+j}[o, i]."""
        wr = wp.tile([128, SC, D], f32, tag="wr")
        for j in range(2):
            nc.sync.dma_start(wr[64 * j:64 * j + 64],
                              w_view[j, :, c * SC:(c + 1) * SC, :])
        shp = (128, SC, D)
        tmp = wp.tile(list(shp), f32, tag="wtmp")
        nc.vector.tensor_tensor(tmp[:], wr[:], diag_m[:, None, :].to_broadcast(shp),
                                op=mybir.AluOpType.mult)
        dsum = wp.tile([128, SC], f32, tag="dsum")
        nc.vector.reduce_sum(dsum[:], tmp[:], axis=mybir.AxisListType.X)
        dexp = wp.tile([128, SC], f32, tag="dexp")
        nc.scalar.activation(dexp[:], dsum[:], mybir.ActivationFunctionType.Exp)
        nc.vector.tensor_tensor(wr[:], wr[:], tril_m[:, None, :].to_broadcast(shp),
                                op=mybir.AluOpType.mult)
        nc.gpsimd.tensor_tensor(tmp[:], diag_m[:, None, :].to_broadcast(shp),
                                dexp[:, :, None].to_broadcast(shp),
                                op=mybir.AluOpType.mult)
        nc.vector.tensor_tensor(wr[:], wr[:], tmp[:], op=mybir.AluOpType.add)

        # Walrus requires transpose outputs at PSUM partition 0, so stage the
        # two 64x64 blocks on the diagonal of a 128x128 tile and do one full
        # transpose; the off-diagonal garbage is never read back.
        lt = lt_pool.tile([128, SC, D], dt_mm, tag="lt")
        for sl in range(SC):
            stage = wp.tile([128, 128], f32, tag="wstage")
            nc.gpsimd.tensor_copy(stage[0:64, 0:64], wr[0:64, sl, :])
            nc.gpsimd.tensor_copy(stage[64:128, 64:128], wr[64:128, sl, :])
            pslt = wpsum.tile([128, 128], f32, tag="pslt")
            nc.tensor.matmul(pslt[:], lhsT=stage[:], rhs=ident[:],
                             is_transpose=True)
            nc.vector.tensor_copy(lt[0:64, sl, :], pslt[0:64, 0:64])
            nc.scalar.copy(lt[64:128, sl, :], pslt[64:128, 64:128])
        lt_tiles.append(lt)
        return lt

    def do_group(g):
        xg = xg_pool.tile([128, NT, CG], dt_t, tag="xg")
        nc.sync.dma_start(xg[:], x_view[:, :, g * CG:(g + 1) * CG])
        yg = yg_pool.tile([128, NT, CG], f32, tag="yg")
        for sl in range(SG):
            s = g * SG + sl
            lt = lt_tiles[s // SC]
            slc = s % SC
            # x chunks -> xT strip [128 (j,i), NS]
            psx = psx_pool.tile([128, NS], dt_t, tag="psx")
            for t in range(NT):
                nc.tensor.matmul(psx[:, t * 128:(t + 1) * 128],
                                 lhsT=xg[:, t, sl * 128:(sl + 1) * 128],
                                 rhs=ident_t[:], is_transpose=True,
                                 start=(t == 0), stop=(t == NT - 1))
            xt = xt_pool.tile([128, NS], dt_mm, tag="xt")
            nc.vector.tensor_copy(xt[:], psx[:])
            # fp32-family matmuls need PSUM partition 0, so use one full
            # 128-contraction matmul with a block-diagonal stationary:
            # bd[64j+i, 64j'+o] = LT[64j+i, o] * blockdiag_mask
            bd = bd_pool.tile([128, 2, 64], dt_mm, tag="bd")
            nc.vector.tensor_tensor(
                bd[:], lt[:, slc, None, :].to_broadcast((128, 2, D)),
                bdm[:], op=mybir.AluOpType.mult)
            psy = psy_pool.tile([128, NS], f32, tag="psy")
            nc.tensor.matmul(psy[:], lhsT=bd.rearrange("p u c -> p (u c)"),
                             rhs=xt[:])
            yt = yt_pool.tile([128, NS], dt_t, tag="yt")
            nc.scalar.copy(yt[:], psy[:])
            # back to natural layout
            psyt = psyt_pool.tile([128, NS], dt_t, tag="psyt")
            for t in range(NT):
                nc.tensor.matmul(psyt[:, t * 128:(t + 1) * 128],
                                 lhsT=yt[:, t * 128:(t + 1) * 128],
                                 rhs=ident_t[:], is_transpose=True,
                                 start=(t == 0), stop=(t == NT - 1))
            dst = yg[:, :, sl * 128:(sl + 1) * 128]
            src = psyt.rearrange("p (t c) -> p t c", c=128)
            if sl % 2 == 0:
                nc.scalar.copy(dst, src)
            else:
                nc.vector.tensor_copy(dst, src)
        nc.sync.dma_start(y_view[:, :, g * CG:(g + 1) * CG], yg[:])

    # interleave weight-chunk prep with the strip groups that consume it
    for _rep in range(repeat):
        lt_tiles.clear()
        for c in range(WC):
            prep_weight_chunk(c)
            for g in range(c * GPC, (c + 1) * GPC):
                do_group(g)


def build(NS=NS, B=B_FULL, SG=16, SC=16, use_f32r=True, f32r_t=False,
          repeat=1):
    key = (NS, B, SG, SC, use_f32r, f32r_t, repeat)
    if key in _built:
        return _built[key]
    import concourse.tile as tile
    import concourse.mybir as mybir
    from concourse import bacc

    f32 = mybir.dt.float32
    dt_x = mybir.dt.float32r if f32r_t else f32
    C = B * D
    nc = bacc.Bacc("TRN2", target_bir_lowering=False, debug=False)
    x_d = nc.dram_tensor("x", [NS, C], dt_x, kind="ExternalInput").ap()
    w_d = nc.dram_tensor("w", [B, D, D], f32, kind="ExternalInput").ap()
    y_d = nc.dram_tensor("y", [NS, C], f32, kind="ExternalOutput").ap()
    with tile.TileContext(nc) as tc, ExitStack() as ctx:
        _body(ctx, tc, y_d, x_d, w_d, NS=NS, B=B, SG=SG, SC=SC,
              use_f32r=use_f32r, f32r_t=f32r_t, repeat=repeat)
    nc.compile()
    _built[key] = nc
    return nc


def _pin_compile_cache(extra=""):
    import hashlib
    with open(os.path.abspath(__file__), "rb") as f:
        h = hashlib.sha256(f.read() + extra.encode()).hexdigest()[:16]
    os.environ["NEURON_COMPILE_CACHE_URL"] = f"/tmp/neuron_cache_{h}"


def run(x, weights, trace=False, use_f32r=True):
    from concourse import bass_utils

    _pin_compile_cache()

    x = np.asarray(x)
    weights = np.asarray(weights)
    assert x.shape == (N_FULL, B_FULL * D), x.shape
    assert weights.shape == (B_FULL, D, D), weights.shape
    x32 = np.ascontiguousarray(x, dtype=np.float32)
    w32 = np.ascontiguousarray(weights, dtype=np.float32)

    nc = build(use_f32r=use_f32r)
    in_maps = [{"x": x32[k * NS:(k + 1) * NS], "w": w32} for k in range(NCORES)]
    res = bass_utils.run_bass_kernel_spmd(
        nc, in_maps, core_ids=list(range(NCORES)), trace=trace)
    y = np.concatenate([res.results[k]["y"] for k in range(NCORES)], axis=0)
    return y.astype(x.dtype, copy=False), res


def kernel(x, weights):
    y, _ = run(x, weights)
    return y
